# revision 16
# baseline (speedup 1.0000x reference)
"""GRU kernel for Trainium2, 8 NeuronCores (SPMD, no collectives).

Strategy:
  - Recurrence (T=128 steps, batch 32, hidden 1024) is replicated on all 8
    cores: the per-step h@W matmuls are weight-streaming-bound on the PE, so
    sharding batch would not reduce per-step time, and every core needs the
    full h history for its slice of the output head anyway.
  - Matmuls run in bf16 (inputs rounded, fp32 PSUM accumulation); the gate
    math / state update stays fp32.
  - The x-side projections (one_hot @ W_x* == embedding row gathers) are
    indirect-DMA gathers from bf16 tables, folded into the PSUM accumulation
    via an extra identity K-tile (psum += I32.T @ x_rows).
  - Output head (h @ W_ho + b_o -> (T*B, 32000)) is vocab-sharded: each core
    computes a (4096, 4000) f32 slice; host concatenates.

Layout notes:
  - h_hist (SBUF, bf16) holds h_t^T for all steps in k-major layout:
    col = k*4128 + slot*32 + b, where slot = t (slot 0 = initial state).
    It doubles as the stationary operand source for both the recurrence
    (slice [k, t, :32]) and the head (slice [k, 4m+1, :128]).
"""

import numpy as np
import ml_dtypes

import concourse.bass as bass
import concourse.mybir as mybir
import concourse.tile as tile
from concourse import bacc
from concourse.bass_utils import run_bass_kernel_spmd
from concourse.masks import make_identity

V, H, B, T = 32000, 1024, 32, 128
NCORES = 8
VS = V // NCORES  # 4000 vocab columns per core
KT = H // 128  # 8 contraction tiles
SLOTS = T + 1
KSTRIDE = SLOTS * B  # 4128
BF = mybir.dt.bfloat16
F32 = mybir.dt.float32
AF = mybir.ActivationFunctionType
OP = mybir.AluOpType

LAST_RESULT = None  # BassKernelResults from the most recent run (for test.py)
_NC_CACHE = {}


def _build_nc():
    nc = bacc.Bacc(None, target_bir_lowering=False)

    tok = nc.dram_tensor("tok", [B, T], mybir.dt.int32, kind="ExternalInput")
    state = nc.dram_tensor("state", [B, H], F32, kind="ExternalInput")
    wrz = nc.dram_tensor("wrz", [H, 2 * H], BF, kind="ExternalInput")
    whh = nc.dram_tensor("whh", [H, H], BF, kind="ExternalInput")
    # interleaved x-side tables: row tok = [W_xr[tok] | W_xz[tok] | W_xh[tok]]
    wxall = nc.dram_tensor("wxall", [V, 3 * H], BF, kind="ExternalInput")
    who = nc.dram_tensor("who", [H, VS], BF, kind="ExternalInput")
    bo = nc.dram_tensor("bo", [128, VS], F32, kind="ExternalInput")
    out = nc.dram_tensor("out", [T * B, VS], F32, kind="ExternalOutput")
    hout = nc.dram_tensor("hout", [B, H], F32, kind="ExternalOutput")

    with tile.TileContext(nc) as tc:
        with tc.tile_pool(name="persist", bufs=1) as pp:
            h_hist = pp.tile([128, KT * KSTRIDE], BF)
            ident = pp.tile([128, 128], BF)
            tok_sb = pp.tile([B, T], mybir.dt.int32)
            make_identity(nc, ident[:, :])
            nc.sync.dma_start(out=tok_sb[:, :], in_=tok[:, :])

            h_hist3 = h_hist.rearrange("p (k n) -> p k n", k=KT)

            def hist_ap(k, slot, width=B):
                base = k * KSTRIDE + slot * B
                return h_hist[:, base : base + width]

            # ---------------- recurrence ----------------
            with (
                tc.tile_pool(name="recw", bufs=1) as rw,
                tc.tile_pool(name="work", bufs=2) as wk,
                tc.tile_pool(name="extp", bufs=3) as extp,
                tc.tile_pool(name="mmp", bufs=4, space="PSUM") as mmp,
                tc.tile_pool(name="tpp", bufs=2, space="PSUM") as tpp,
            ):
                wrz_sb = rw.tile([128, KT * 2 * H], BF)
                whh_sb = rw.tile([128, KT * H], BF)
                nc.sync.dma_start(
                    out=wrz_sb.rearrange("p (k n) -> p k n", k=KT),
                    in_=wrz.rearrange("(k p) n -> p k n", p=128),
                )
                nc.sync.dma_start(
                    out=whh_sb.rearrange("p (k n) -> p k n", k=KT),
                    in_=whh.rearrange("(k p) n -> p k n", p=128),
                )

                def store_hT(src_bf, slot):
                    # transpose (32, 1024) -> 8x (128, 32) into one PSUM tile,
                    # then one strided DVE copy into h_hist[:, k, slot*32:+32]
                    tp = tpp.tile([128, KT * B], BF, tag="tp", name="tp")
                    for k in range(KT):
                        nc.tensor.transpose(
                            tp[:, k * B : (k + 1) * B],
                            src_bf[:, k * 128 : (k + 1) * 128],
                            ident[:B, :B],
                        )
                    dst = h_hist3[:, :, slot * B : (slot + 1) * B]
                    src = tp.rearrange("p (k b) -> p k b", k=KT)
                    nc.vector.tensor_copy(dst, src)

                h_f = wk.tile([B, H], F32, tag="h", bufs=2, name="h_f")
                nc.sync.dma_start(out=h_f[:, :], in_=state[:, :])
                h_b = wk.tile([B, H], BF, tag="hb", bufs=2, name="h_b")
                nc.vector.tensor_copy(h_b[:, :], h_f[:, :])
                store_hT(h_b, 0)

                for t in range(T):
                    ext = extp.tile([B, 3 * H], BF, tag="ext", name="ext")
                    nc.gpsimd.indirect_dma_start(
                        out=ext[:, :],
                        out_offset=None,
                        in_=wxall[:, :],
                        in_offset=bass.IndirectOffsetOnAxis(
                            ap=tok_sb[:, t : t + 1], axis=0
                        ),
                    )

                    r_f = wk.tile([B, H], F32, tag="r", name="r_f")
                    z_f = wk.tile([B, H], F32, tag="z", name="z_f")
                    for c in range(4):  # r0 r1 z0 z1 chunks of 512
                        ps = mmp.tile([B, 512], F32, tag="mm", name="ps_rz")
                        for k in range(KT):
                            nc.tensor.matmul(
                                ps[:, :],
                                hist_ap(k, t),
                                wrz_sb[:, k * 2 * H + c * 512 : k * 2 * H + (c + 1) * 512],
                                start=(k == 0),
                                stop=False,
                            )
                        nc.tensor.matmul(
                            ps[:, :],
                            ident[:B, :B],
                            ext[:, c * 512 : (c + 1) * 512],
                            start=False,
                            stop=True,
                        )
                        tgt = r_f if c < 2 else z_f
                        nc.scalar.activation(
                            tgt[:, (c % 2) * 512 : (c % 2 + 1) * 512],
                            ps[:, :],
                            AF.Sigmoid,
                        )

                    rh_b = wk.tile([B, H], BF, tag="rhb", name="rh_b")
                    nc.vector.tensor_tensor(
                        out=rh_b[:, :], in0=r_f[:, :], in1=h_f[:, :], op=OP.mult
                    )
                    tp2 = tpp.tile([128, KT * B], BF, tag="tp", name="tp2")
                    for k in range(KT):
                        nc.tensor.transpose(
                            tp2[:, k * B : (k + 1) * B],
                            rh_b[:, k * 128 : (k + 1) * 128],
                            ident[:B, :B],
                        )
                    rhT = wk.tile([128, KT * B], BF, tag="rhT", name="rhT")
                    nc.vector.tensor_copy(rhT[:, :], tp2[:, :])

                    c_f = wk.tile([B, H], F32, tag="c", name="c_f")
                    for c2 in range(2):
                        ps = mmp.tile([B, 512], F32, tag="mm", name="ps_hh")
                        for k in range(KT):
                            nc.tensor.matmul(
                                ps[:, :],
                                rhT[:, k * B : (k + 1) * B],
                                whh_sb[:, k * H + c2 * 512 : k * H + (c2 + 1) * 512],
                                start=(k == 0),
                                stop=False,
                            )
                        nc.tensor.matmul(
                            ps[:, :],
                            ident[:B, :B],
                            ext[:, 2 * H + c2 * 512 : 2 * H + (c2 + 1) * 512],
                            start=False,
                            stop=True,
                        )
                        nc.scalar.activation(
                            c_f[:, c2 * 512 : (c2 + 1) * 512], ps[:, :], AF.Tanh
                        )

                    # h_new = c + z * (h - c)
                    t1 = wk.tile([B, H], F32, tag="t1", name="t1")
                    nc.vector.tensor_tensor(
                        out=t1[:, :], in0=h_f[:, :], in1=c_f[:, :], op=OP.subtract
                    )
                    t2 = wk.tile([B, H], F32, tag="t2", name="t2")
                    nc.vector.tensor_tensor(
                        out=t2[:, :], in0=z_f[:, :], in1=t1[:, :], op=OP.mult
                    )
                    h_new = wk.tile([B, H], F32, tag="h", bufs=2, name="h_new")
                    nc.vector.tensor_tensor(
                        out=h_new[:, :], in0=c_f[:, :], in1=t2[:, :], op=OP.add
                    )
                    h_nb = wk.tile([B, H], BF, tag="hb", bufs=2, name="h_nb")
                    nc.vector.tensor_copy(h_nb[:, :], h_new[:, :])
                    store_hT(h_nb, t + 1)
                    h_f = h_new

                nc.sync.dma_start(out=hout[:, :], in_=h_f[:, :])

            # ---------------- output head ----------------
            with (
                tc.tile_pool(name="headw", bufs=1) as hw,
                tc.tile_pool(name="outp", bufs=2) as hop,
                tc.tile_pool(name="hps", bufs=4, space="PSUM") as hps,
            ):
                who_sb = hw.tile([128, KT * VS], BF)
                bo_sb = hw.tile([128, VS], F32)
                nc.sync.dma_start(out=bo_sb[:, :], in_=bo[:, :])
                nc.sync.dma_start(
                    out=who_sb.rearrange("p (k n) -> p k n", k=KT),
                    in_=who.rearrange("(k p) n -> p k n", p=128),
                )

                NB = 8
                CH = VS // NB  # 500
                for m in range(T * B // 128):  # 32 tiles of 128 output rows
                    ob = hop.tile([128, VS], F32, tag="ob", name="ob")
                    for nb in range(NB):
                        ps = hps.tile([128, CH], F32, tag="hmm", name="ps_head")
                        for k in range(KT):
                            nc.tensor.matmul(
                                ps[:, :],
                                hist_ap(k, 4 * m + 1, width=128),
                                who_sb[:, k * VS + nb * CH : k * VS + (nb + 1) * CH],
                                start=(k == 0),
                                stop=(k == KT - 1),
                            )
                        nc.vector.tensor_tensor(
                            out=ob[:, nb * CH : (nb + 1) * CH],
                            in0=ps[:, :],
                            in1=bo_sb[:, nb * CH : (nb + 1) * CH],
                            op=OP.add,
                        )
                    nc.sync.dma_start(
                        out=out[m * 128 : (m + 1) * 128, :], in_=ob[:, :]
                    )

    return nc


def _get_nc():
    if "nc" not in _NC_CACHE:
        nc = _build_nc()
        if not nc.is_finalized():
            nc.finalize()
        _NC_CACHE["nc"] = nc
    return _NC_CACHE["nc"]


def prepare_in_maps(x, state, W_xr, W_hr, b_r, W_xz, W_hz, b_z, W_xh, W_hh, b_h, W_ho, b_o):
    bf = ml_dtypes.bfloat16
    x = np.asarray(x)
    state = np.asarray(state)

    tok = np.ascontiguousarray(x.astype(np.int32))
    state_f = np.ascontiguousarray(np.asarray(state, dtype=np.float32))
    wrz = np.ascontiguousarray(
        np.concatenate([np.asarray(W_hr), np.asarray(W_hz)], axis=1)
    ).astype(bf)
    whh = np.ascontiguousarray(np.asarray(W_hh)).astype(bf)
    # fold per-gate biases into the gather tables; interleave row-wise
    wxall = np.empty((V, 3 * H), dtype=bf)
    wxall[:, 0:H] = (np.asarray(W_xr) + np.asarray(b_r)[None, :]).astype(bf)
    wxall[:, H : 2 * H] = (np.asarray(W_xz) + np.asarray(b_z)[None, :]).astype(bf)
    wxall[:, 2 * H : 3 * H] = (np.asarray(W_xh) + np.asarray(b_h)[None, :]).astype(bf)
    who_f = np.asarray(W_ho)
    bo_f = np.asarray(b_o, dtype=np.float32)

    in_maps = []
    for c in range(NCORES):
        sl = slice(c * VS, (c + 1) * VS)
        in_maps.append(
            dict(
                tok=tok,
                state=state_f,
                wrz=wrz,
                whh=whh,
                wxall=wxall,
                who=np.ascontiguousarray(who_f[:, sl]).astype(bf),
                bo=np.ascontiguousarray(
                    np.broadcast_to(bo_f[sl][None, :], (128, VS))
                ),
            )
        )
    return in_maps


def kernel(x, state, W_xr, W_hr, b_r, W_xz, W_hz, b_z, W_xh, W_hh, b_h, W_ho, b_o):
    global LAST_RESULT
    in_maps = prepare_in_maps(
        x, state, W_xr, W_hr, b_r, W_xz, W_hz, b_z, W_xh, W_hh, b_h, W_ho, b_o
    )
    nc = _get_nc()
    res = run_bass_kernel_spmd(nc, in_maps, core_ids=list(range(NCORES)))
    LAST_RESULT = res
    logits = np.concatenate([r["out"] for r in res.results], axis=1)
    return logits, res.results[0]["hout"]
